# revision 2
# baseline (speedup 1.0000x reference)
"""Trainium2 Bass kernel v3 for 2-layer LSTM dynamics (B=64, T=512, D=64, H=512, out=32).

Sharding: data-parallel over batch across 8 cores (BL=8), weights replicated.

Key structure vs baseline:
  - Interleaved recurrences: layer-1 runs DELAY steps behind layer-0 in the
    emission order, so each layer's activation chain hides under the other
    layer's 64 LDWEIGHTS+MATMUL block (PE never idles on the chain).
  - W_hh in fp8 e3m4 (scale 256): FWL loads 4B/cycle -> half the LDW time
    of bf16. rhs (h) stays fp16. Descale fused into the psum+xg add
    (affine_then_add, one DVE op).
  - One [128,128] gate tile per step (col = gate*32 + k*8 + b), so the
    activation pipeline is 3 ACT + 5 DVE instructions per step instead of
    12 ACT + 20 DVE.
  - All 16-bit tensors fp16 (not bf16) for ~8x less rounding error.
  - xg precompute for layer-1 is emitted per 64-step chunk as layer-0
    completes it, and fills PE gaps via Tile's scheduler.
"""

import numpy as np
import ml_dtypes

import concourse.bass as bass
import concourse.mybir as mybir
import concourse.tile as tile
from concourse.alu_op_type import AluOpType
from concourse.bass_utils import run_bass_kernel_spmd

# ---------------------------------------------------------------------------
# walrus workaround: split the final TileContext drain (multi-sem-wait CTRL
# instruction) into one drain per proc; installed walrus caps waits at 1.
from concourse.vector_clock import ScopedClock, VectorClock


def _drain_and_barrier_split(self, tick_clock, wait_clock):
    gc = tick_clock.global_clock
    n = len(gc)
    emitted = 0
    for p in range(n):
        if gc[p] > 0:
            v = [0] * n
            v[p] = gc[p]
            d = self.nc.sync.drain()
            wait_clock.add_sem_waits(d.ins, ScopedClock({None: VectorClock(v)}))
            emitted += 1
    if emitted == 0:
        self.nc.sync.drain()
    self.nc.all_engine_barrier()
    assert self.sems is not None
    popped = self.nc._tile_sem_poison_stack.pop()
    assert popped is self._sem_poison
    self.nc.clear_and_free_semaphores(list(self.sems.allocated().values()))
    self.nc.all_engine_barrier()


tile.TileContext._drain_and_barrier = _drain_and_barrier_split

import bass_rust

_wsplit_ctr = [0]


def _split_multi_waits(nc):
    """walrus also caps waits at 1 on regular instructions: move extra waits
    onto same-engine NoOps inserted immediately before."""
    for fn in nc.m.functions:
        for blk in fn.blocks:
            insts = blk.instructions
            i = 0
            while i < len(insts):
                inst = insts[i]
                si = inst.sync_info
                if si is not None and len(si.on_wait) > 1:
                    waits = list(si.on_wait)
                    si.on_wait = [waits[-1]]
                    for w in waits[:-1]:
                        _wsplit_ctr[0] += 1
                        no = mybir.InstNoOp(
                            name=f"wsplit_{_wsplit_ctr[0]}", ins=[], outs=[])
                        no.engine = inst.engine
                        no.sync_info = bass_rust.SyncInfo(
                            on_wait=[w], on_update=[])
                        insts.insert(i, no)
                        i += 1
                i += 1
# ---------------------------------------------------------------------------

COALESCE_PE_INCS = True


def _coalesce_pe_incs(nc):
    """Coalesce runs of wait-free PE instructions' sem-inc(+1) updates into a
    single sem-add-imm on the last updating instruction of the run.

    The PE clock semaphore is incremented by every Matmult (~26ns serialized
    EVT_SEM write each); consumers only ever wait for step-boundary values, so
    deferring intermediate increments to the next wait boundary removes ~63
    sem writes per 64-matmul block. Runs contain only wait-free instructions,
    so once entered they always retire -> the summed increment always fires;
    any instruction carrying a wait flushes pending increments onto its
    predecessor first, so the PE semaphore is fully up to date before any PE
    stall. Breaks on: waits, non-single-inc updates, sem-id changes.
    """
    PE = mybir.EngineType.PE
    n_removed = [0]
    for fn in nc.m.functions:
        for blk in fn.blocks:
            pending = 0          # deferred increments
            carrier = None       # last instr in run still holding an update
            carrier_sem = None
            def flush():
                nonlocal pending, carrier, carrier_sem
                if pending and carrier is not None:
                    si = carrier.sync_info
                    u = si.on_update[0]
                    nu = bass_rust.SyncUpdate(
                        sync_type=u.sync_type, id=u.id, ant_name=u.ant_name,
                        update_mode="sem-add-imm",
                        update_value=u.update_value + pending,
                        update_reg=None)
                    si.on_update = [nu]
                    carrier.sync_info = si
                pending = 0; carrier = None; carrier_sem = None
            for inst in blk.instructions:
                if inst.engine != PE:
                    continue
                si = inst.sync_info
                w = list(si.on_wait) if si else []
                u = list(si.on_update) if si else []
                simple_inc = (
                    len(u) == 1 and u[0].update_mode == "sem-inc"
                    and u[0].update_value == 1)
                if w:
                    flush()
                    continue       # keep its own update untouched
                if not u:
                    continue       # neutral (Ldweights): stays inside run
                if not simple_inc or (carrier_sem is not None
                                      and u[0].id != carrier_sem):
                    flush()
                    if not simple_inc:
                        continue
                # defer previous carrier's inc onto this one
                if carrier is not None:
                    csi = carrier.sync_info
                    csi.on_update = []
                    carrier.sync_info = csi
                    pending += 1
                    n_removed[0] += 1
                carrier = inst
                carrier_sem = u[0].id
            flush()
    return n_removed[0]


F32 = mybir.dt.float32
F16 = mybir.dt.float16
FP8 = mybir.dt.float8e3
AF = mybir.ActivationFunctionType
ALU = None  # set lazily

USE_FP8_WHH = True
FP8_SCALE = 256.0

B, D_IN, H, D_OUT = 64, 64, 512, 32
G = 4 * H          # 2048 gate rows
BL = 8             # batch per core
NCORES = 8
KT = H // 128      # 4 hidden chunks
MT = G // 128      # 16 gate tiles (m = gate*KT + k)
DELAY = 72         # layer-1 emission lag (steps); > 64 + slack


def build_kernel(T):
    nc = bass.Bass()
    whh_dt = FP8 if USE_FP8_WHH else F16
    inv_s = (1.0 / FP8_SCALE) if USE_FP8_WHH else 1.0

    xT_d = nc.declare_dram_parameter("xT", [D_IN, T * BL], F16, isOutput=False)
    wih0_d = nc.declare_dram_parameter("Wih0T", [D_IN, G], F16, isOutput=False)
    whh0_d = nc.declare_dram_parameter("Whh0T", [H, G], whh_dt, isOutput=False)
    wih1_d = nc.declare_dram_parameter("Wih1T", [H, G], F16, isOutput=False)
    whh1_d = nc.declare_dram_parameter("Whh1T", [H, G], whh_dt, isOutput=False)
    wout_d = nc.declare_dram_parameter("WoutT", [H, D_OUT], F16, isOutput=False)
    b0_d = nc.declare_dram_parameter("b0", [1, G], F16, isOutput=False)
    b1_d = nc.declare_dram_parameter("b1", [1, G], F16, isOutput=False)
    bout_d = nc.declare_dram_parameter("bout", [D_OUT, 1], F32, isOutput=False)
    y_d = nc.declare_dram_parameter("yT", [D_OUT, BL], F32, isOutput=True)

    # xg staging in DRAM, step-major: [t, p, col] with col = gate*32 + k*8 + b
    xg0_d = nc.dram_tensor("xg0", [T, 128, 128], F16)
    xg1_d = nc.dram_tensor("xg1", [T, 128, 128], F16)

    NTOK = T * BL              # tokens per core
    NCH = min(512, NTOK)       # precompute free-dim chunk (tokens)
    n_chunks = NTOK // NCH
    SPC = NCH // BL            # steps per chunk (64)

    with tile.TileContext(nc) as tc:
        with (
            tc.tile_pool(name="w", bufs=1) as wpool,
            tc.tile_pool(name="xg", bufs=8) as xg_pool,
            tc.tile_pool(name="g", bufs=6) as g_pool,
            tc.tile_pool(name="sa", bufs=6) as sa_pool,
            tc.tile_pool(name="tmp", bufs=6) as tmp_pool,
            tc.tile_pool(name="pre", bufs=3) as pre_pool,
            tc.tile_pool(name="psum_rec", bufs=5, space="PSUM") as psum_rec,
            tc.tile_pool(name="psum_pre", bufs=3, space="PSUM") as psum_pre,
        ):
            # ---- load weights / persistent state ----
            xT = wpool.tile([D_IN, NTOK], F16, name="xT", tag="xT")
            nc.sync.dma_start(xT[:], xT_d[:])
            wih0 = wpool.tile([D_IN, G], F16, name="wih0", tag="wih0")
            nc.sync.dma_start(wih0[:], wih0_d[:])
            whh0 = [wpool.tile([128, G], whh_dt, name=f"whh0_{k}", tag=f"whh0_{k}") for k in range(KT)]
            wih1 = [wpool.tile([128, G], F16, name=f"wih1_{k}", tag=f"wih1_{k}") for k in range(KT)]
            whh1 = [wpool.tile([128, G], whh_dt, name=f"whh1_{k}", tag=f"whh1_{k}") for k in range(KT)]
            wout = [wpool.tile([128, D_OUT], F16, name=f"wout_{k}", tag=f"wout_{k}") for k in range(KT)]
            for k in range(KT):
                sl = slice(128 * k, 128 * (k + 1))
                nc.sync.dma_start(whh0[k][:], whh0_d[sl, :])
                nc.sync.dma_start(wih1[k][:], wih1_d[sl, :])
                nc.sync.dma_start(whh1[k][:], whh1_d[sl, :])
                nc.sync.dma_start(wout[k][:], wout_d[sl, :])
            b0 = wpool.tile([1, G], F16, name="b0", tag="b0")
            nc.sync.dma_start(b0[:], b0_d[:])
            b1 = wpool.tile([1, G], F16, name="b1", tag="b1")
            nc.sync.dma_start(b1[:], b1_d[:])
            bout = wpool.tile([D_OUT, 1], F32, name="bout", tag="bout")
            nc.sync.dma_start(bout[:], bout_d[:])

            ones = wpool.tile([1, NCH], F16, name="ones", tag="ones")
            nc.gpsimd.memset(ones[:], 1.0)
            z8 = wpool.tile([128, BL], F16, name="z8", tag="z8")
            nc.gpsimd.memset(z8[:], 0.0)

            # h storage: layer0 full sequence [128, (k, t, b)] fp16
            h0seq = wpool.tile([128, KT * T * BL], F16, name="h0seq", tag="h0seq")
            # layer1 ring [128, (slot, k, b)] fp16
            h1r = wpool.tile([128, 2 * KT * BL], F16, name="h1r", tag="h1r")
            c0 = wpool.tile([128, KT * BL], F32, name="c0", tag="c0")
            nc.gpsimd.memset(c0[:], 0.0)
            c1 = wpool.tile([128, KT * BL], F32, name="c1", tag="c1")
            nc.gpsimd.memset(c1[:], 0.0)

            def h0_rhs(t, kk):      # [128, 8] rhs slice of layer-0 h at step t
                return h0seq[:, kk * (T * BL) + t * BL: kk * (T * BL) + (t + 1) * BL]

            def h1_rhs(t, kk):
                s = (t % 2) * KT * BL
                return h1r[:, s + kk * BL: s + kk * BL + BL]

            def h0_dst(t):          # strided [128, k:4, b:8] write for step t
                return h0seq[:].rearrange("p (k tb) -> p k tb", k=KT)[:, :, t * BL:(t + 1) * BL]

            def h1_dst(t):
                s = (t % 2) * KT * BL
                return h1r[:, s: s + KT * BL]

            # ---- xg precompute: out_dram[t,p,col] = (W_ih @ x + b).T ----
            def emit_xg_chunk(lhs_tiles, rhs_src, bias, out_dram, c):
                csl = slice(c * NCH, (c + 1) * NCH)
                t0 = c * SPC
                for m in range(MT):
                    gate, k = divmod(m, KT)
                    msl = slice(m * 128, (m + 1) * 128)
                    ps = psum_pre.tile([128, NCH], F32, name="pre", tag="pre")
                    for kk in range(len(lhs_tiles)):
                        nc.tensor.matmul(
                            ps[:], lhs_tiles[kk][:, msl], rhs_src(kk, csl),
                            start=(kk == 0), stop=False,
                        )
                    nc.tensor.matmul(
                        ps[:], bias[0:1, msl], ones[0:1, :],
                        start=False, stop=True,
                    )
                    xgsb = pre_pool.tile([128, NCH], F16, name="xgsb", tag="xgsb")
                    nc.vector.tensor_copy(xgsb[:], ps[:])
                    col0 = gate * 32 + k * 8
                    nc.sync.dma_start(
                        out_dram[t0:t0 + SPC, :, col0:col0 + 8].rearrange(
                            "t p b -> p t b"),
                        xgsb[:].rearrange("p (t b) -> p t b", b=BL),
                    )

            # ---- one recurrence step ----
            def emit_step(layer, t):
                whh = whh0 if layer == 0 else whh1
                xg_d = xg0_d if layer == 0 else xg1_d
                c_sb = c0 if layer == 0 else c1
                rhs = (lambda kk: z8[:]) if t == 0 else (
                    (lambda kk: h0_rhs(t - 1, kk)) if layer == 0
                    else (lambda kk: h1_rhs(t - 1, kk)))

                xg_t = xg_pool.tile([128, 128], F16, name="xg", tag="xg")
                nc.sync.dma_start(xg_t[:], xg_d[t])

                ps = psum_rec.tile([128, 128], F32, name="gps", tag="gps")
                for m in range(MT):
                    gate, k = divmod(m, KT)
                    col0 = gate * 32 + k * 8
                    for kk in range(KT):
                        nc.tensor.matmul(
                            ps[:, col0:col0 + 8],
                            whh[kk][:, m * 128:(m + 1) * 128],
                            rhs(kk),
                            start=(kk == 0), stop=(kk == KT - 1),
                        )
                # g = psum/S + xg   (fp16 out, one standard DVE op)
                g = g_pool.tile([128, 128], F16, name="g", tag="g")
                nc.vector.scalar_tensor_tensor(
                    g[:], ps[:], inv_s, xg_t[:], AluOpType.mult, AluOpType.add)
                # activations: [i f o | g] = sigmoid on 0:96, tanh on 96:128
                sa = sa_pool.tile([128, 128], F16, name="sa", tag="sa")
                nc.scalar.activation(sa[:, 0:96], g[:, 0:96], AF.Sigmoid)
                nc.scalar.activation(sa[:, 96:128], g[:, 96:128], AF.Tanh)
                # c = sigm(f)*c + sigm(i)*tanh(g);  h = sigm(o)*tanh(c)
                # (c *= sigm(f) first: it only needs the sigmoid, so DVE runs
                #  it concurrently with the tanh activation)
                tmp = tmp_pool.tile([128, KT * BL], F32, name="tmp", tag="tmp")
                nc.vector.tensor_mul(c_sb[:], c_sb[:], sa[:, 32:64])
                nc.vector.tensor_mul(tmp[:], sa[:, 0:32], sa[:, 96:128])
                nc.vector.tensor_add(c_sb[:], c_sb[:], tmp[:])
                tc_ = tmp_pool.tile([128, KT * BL], F16, name="tc", tag="tc")
                nc.scalar.activation(tc_[:], c_sb[:], AF.Tanh)
                dst = h0_dst(t) if layer == 0 else h1_dst(t)
                nc.vector.tensor_mul(dst, sa[:, 64:96], tc_[:])

            # ---- emission: xg0 fully, then interleaved recurrences ----
            for c in range(n_chunks):
                emit_xg_chunk([wih0], lambda kk, csl: xT[:, csl], b0, xg0_d, c)

            for e in range(T + DELAY):
                if e < T:
                    emit_step(0, e)
                    if e % SPC == SPC - 1:
                        emit_xg_chunk(
                            wih1,
                            lambda kk, csl: h0seq[:, kk * NTOK:(kk + 1) * NTOK][:, csl],
                            b1, xg1_d, e // SPC)
                if e >= DELAY:
                    emit_step(1, e - DELAY)

            # ---- output: y.T = W_out @ h1_last.T + b_out ----
            ps_y = psum_rec.tile([D_OUT, BL], F32, name="gps", tag="gps")
            for kk in range(KT):
                nc.tensor.matmul(
                    ps_y[:], wout[kk][:], h1_rhs(T - 1, kk),
                    start=(kk == 0), stop=(kk == KT - 1),
                )
            y_sb = g_pool.tile([D_OUT, BL], F32, name="y_sb", tag="y_sb")
            nc.scalar.activation(y_sb[:], ps_y[:], AF.Identity, bias=bout[:, 0:1])
            nc.sync.dma_start(y_d[:], y_sb[:])

    if COALESCE_PE_INCS:
        n = _coalesce_pe_incs(nc)
        print(f"[kernel_v3] coalesced {n} PE sem incs")
    _split_multi_waits(nc)
    return nc


_NC_CACHE = {}


def _get_nc(T):
    if T not in _NC_CACHE:
        _NC_CACHE[T] = build_kernel(T)
    return _NC_CACHE[T]


GATE_PERM = [0, 1, 3, 2]  # torch [i,f,g,o] -> [i, f, o, g]


def _gperm(W):
    return np.ascontiguousarray(
        W.reshape(4, H, *W.shape[1:])[GATE_PERM].reshape(W.shape))


def _whh_conv(W):
    WT = np.ascontiguousarray(W.T)
    if USE_FP8_WHH:
        return (WT * FP8_SCALE).astype(ml_dtypes.float8_e3m4)
    return WT.astype(np.float16)


def _prep_inputs(x, W_ih0, W_hh0, b_ih0, b_hh0, W_ih1, W_hh1, b_ih1, b_hh1,
                 W_out, b_out):
    T = x.shape[1]
    shared = {
        "Wih0T": np.ascontiguousarray(_gperm(W_ih0).T).astype(np.float16),
        "Whh0T": _whh_conv(_gperm(W_hh0)),
        "Wih1T": np.ascontiguousarray(_gperm(W_ih1).T).astype(np.float16),
        "Whh1T": _whh_conv(_gperm(W_hh1)),
        "WoutT": np.ascontiguousarray(W_out.T).astype(np.float16),
        "b0": _gperm((b_ih0 + b_hh0).reshape(G, 1)).reshape(1, G).astype(np.float16),
        "b1": _gperm((b_ih1 + b_hh1).reshape(G, 1)).reshape(1, G).astype(np.float16),
        "bout": b_out.reshape(D_OUT, 1).astype(np.float32),
    }
    in_maps = []
    for c in range(NCORES):
        xc = x[c * BL:(c + 1) * BL]            # [8, T, 64]
        xT = np.ascontiguousarray(xc.transpose(2, 1, 0).reshape(D_IN, T * BL))
        in_maps.append({"xT": xT.astype(np.float16), **shared})
    return in_maps


def kernel(x, W_ih0, W_hh0, b_ih0, b_hh0, W_ih1, W_hh1, b_ih1, b_hh1,
           W_out, b_out):
    T = x.shape[1]
    nc = _get_nc(T)
    in_maps = _prep_inputs(x, W_ih0, W_hh0, b_ih0, b_hh0, W_ih1, W_hh1,
                           b_ih1, b_hh1, W_out, b_out)
    res = run_bass_kernel_spmd(nc, in_maps, core_ids=list(range(NCORES)))
    out = np.concatenate(
        [res.results[c]["yT"].T for c in range(NCORES)], axis=0)
    return np.ascontiguousarray(out.astype(np.float32))
